# revision 1
# baseline (speedup 1.0000x reference)
"""Trainium2 Bass kernel for a dense cross-task transformer block.

Math notes
----------
The reference "attention" has sequence length 1 on the key axis, so
softmax(scores) == 1.0 exactly and the whole q/k/score path is dead:

    mha_len1(q_in, kv_in, ...) == (kv_in @ wv.T + bv) @ wo.T + bo

which folds (on host) into a single matmul with W = wo @ wv and
b = wo @ bv + bo.  The block is then:

    verb1 = LN(verb + noun @ W1.T + c1)          (ln_v)
    verb2 = verb1 + FFN_v(verb1)
    noun1 = LN(noun + verb2 @ W2.T + c2)         (ln_n)
    noun2 = noun1 + FFN_n(noun1)
    return verb2, noun2

Device strategy
---------------
Pure data parallel over 8 cores (batch 16384 -> 2048 rows/core), weights
replicated.  On device everything is kept feature-major ([E, batch]) so
every matmul contracts along the SBUF partition dim.  Matmuls run as
float32r (1 cycle/row for N>=256).  LayerNorm reduces across partitions
via ones-vector matmuls; stats are broadcast back across partitions with
K=1 matmuls.  The second FFN matmul runs in bf16 (hidden activations and
w2 weights) - the result only feeds a residual delta, so precision loss
is negligible.
"""

import numpy as np
import ml_dtypes
from contextlib import ExitStack

import concourse.bass as bass
import concourse.bacc as bacc_mod
import concourse.mybir as mybir
import concourse.tile as tile
from concourse.bass_utils import run_bass_kernel_spmd

E = 1024          # embed dim
H2 = 2048         # FFN hidden dim
B_TOTAL = 16384
NCORES = 8
B = B_TOTAL // NCORES   # 2048 rows per core
P = 128
EPS = 1e-5
CHUNK = 512       # attn/LN phase column chunk
NCHUNKS = B // CHUNK
KT = E // P       # 8  k-tiles over E
MT = E // P       # 8  m-tiles over E
HT = H2 // P      # 16 tiles over hidden

F32 = mybir.dt.float32
F32R = mybir.dt.float32r
BF16 = mybir.dt.bfloat16
AF = mybir.ActivationFunctionType
OP = mybir.AluOpType


def _load_pvec(nc, pool, dram_ap, ntiles, tag):
    """DRAM [ntiles*128] vector -> SBUF [128, ntiles], element (p,t) = v[t*128+p]."""
    t = pool.tile([P, ntiles], F32, tag=tag, name=tag)
    nc.sync.dma_start(out=t[:], in_=dram_ap.rearrange("(t p) -> p t", p=P))
    return t


def _build_program():
    nc = bacc_mod.Bacc("TRN2", target_bir_lowering=False)

    vT = nc.declare_dram_parameter("vT", [E, B], F32, isOutput=False)
    nT = nc.declare_dram_parameter("nT", [E, B], F32, isOutput=False)
    wvo1 = nc.declare_dram_parameter("wvo1", [E, E], F32, isOutput=False)     # (wo@wv).T : [k, m]
    bvo1 = nc.declare_dram_parameter("bvo1", [E], F32, isOutput=False)
    wvo2 = nc.declare_dram_parameter("wvo2", [E, E], F32, isOutput=False)
    bvo2 = nc.declare_dram_parameter("bvo2", [E], F32, isOutput=False)
    lnvg = nc.declare_dram_parameter("lnvg", [E], F32, isOutput=False)
    lnvb = nc.declare_dram_parameter("lnvb", [E], F32, isOutput=False)
    lnng = nc.declare_dram_parameter("lnng", [E], F32, isOutput=False)
    lnnb = nc.declare_dram_parameter("lnnb", [E], F32, isOutput=False)
    w1v = nc.declare_dram_parameter("w1v", [E, H2], F32, isOutput=False)      # fv_w1.T
    b1v = nc.declare_dram_parameter("b1v", [H2], F32, isOutput=False)
    w2v = nc.declare_dram_parameter("w2v", [H2, E], BF16, isOutput=False)     # fv_w2.T in bf16
    b2v = nc.declare_dram_parameter("b2v", [E], F32, isOutput=False)
    w1n = nc.declare_dram_parameter("w1n", [E, H2], F32, isOutput=False)
    b1n = nc.declare_dram_parameter("b1n", [H2], F32, isOutput=False)
    w2n = nc.declare_dram_parameter("w2n", [H2, E], BF16, isOutput=False)
    b2n = nc.declare_dram_parameter("b2n", [E], F32, isOutput=False)
    ones_d = nc.declare_dram_parameter("ones_d", [P, 1], F32, isOutput=False)
    verb_out = nc.declare_dram_parameter("verb_out", [E, B], F32, isOutput=True)
    noun_out = nc.declare_dram_parameter("noun_out", [E, B], F32, isOutput=True)

    with tile.TileContext(nc) as tc, ExitStack() as ctx:
        const = ctx.enter_context(tc.tile_pool(name="const", bufs=1))
        resid = ctx.enter_context(tc.tile_pool(name="resid", bufs=1))

        ones_col = const.tile([P, 1], F32R, tag="ones_col", name="ones_col")
        nc.sync.dma_start(out=ones_col[:], in_=ones_d[:, :].bitcast(F32R))
        ones_row = const.tile([1, P], F32, tag="ones_row", name="ones_row")
        nc.vector.memset(ones_row[:], 1.0)
        eps_t = const.tile([1, 1], F32, tag="eps", name="eps")
        nc.vector.memset(eps_t[:], EPS)

        bvo1_pb = _load_pvec(nc, const, bvo1[:], MT, "bvo1")
        bvo2_pb = _load_pvec(nc, const, bvo2[:], MT, "bvo2")
        lnvg_pb = _load_pvec(nc, const, lnvg[:], MT, "lnvg")
        lnvb_pb = _load_pvec(nc, const, lnvb[:], MT, "lnvb")
        lnng_pb = _load_pvec(nc, const, lnng[:], MT, "lnng")
        lnnb_pb = _load_pvec(nc, const, lnnb[:], MT, "lnnb")
        b1v_pb = _load_pvec(nc, const, b1v[:], HT, "b1v")
        b2v_pb = _load_pvec(nc, const, b2v[:], MT, "b2v")
        b1n_pb = _load_pvec(nc, const, b1n[:], HT, "b1n")
        b2n_pb = _load_pvec(nc, const, b2n[:], MT, "b2n")

        # persistent residual-stream tiles ([128, B] f32); verb1 in phases
        # A/B, overwritten as noun1 in phases C/D (same tags -> same slots)
        def resid_tiles():
            return [resid.tile([P, B], F32R, tag=f"r{m}", name=f"r{m}") for m in range(MT)]

        def attn_ln_phase(sfx, kxn_dram, res_dram, w_dram, bias_pb, g_pb, b_pb):
            """out_tiles[m][:, :] = LN(res + kxn.T @ w + bias) feature-major."""
            out_tiles = resid_tiles()
            with ExitStack() as pctx:
                wpool = pctx.enter_context(tc.tile_pool(name=f"wv{sfx}", bufs=1))
                kxp = pctx.enter_context(tc.tile_pool(name=f"kx{sfx}", bufs=1))
                vp = pctx.enter_context(tc.tile_pool(name=f"vp{sfx}", bufs=2))
                sqp = pctx.enter_context(tc.tile_pool(name=f"sq{sfx}", bufs=2))
                sm = pctx.enter_context(tc.tile_pool(name=f"sm{sfx}", bufs=1))
                aps = pctx.enter_context(
                    tc.tile_pool(name=f"aps{sfx}", bufs=2, space="PSUM"))
                stp = pctx.enter_context(
                    tc.tile_pool(name=f"st{sfx}", bufs=1, space="PSUM"))
                bcp = pctx.enter_context(
                    tc.tile_pool(name=f"bc{sfx}", bufs=1, space="PSUM"))

                w_tiles = []
                for k in range(KT):
                    wt = wpool.tile([P, E], F32R, tag=f"w{k}", name=f"w{k}")
                    nc.sync.dma_start(out=wt[:], in_=w_dram[k * P:(k + 1) * P, :].bitcast(F32R))
                    w_tiles.append(wt)

                for c in range(NCHUNKS):
                    cs = slice(c * CHUNK, (c + 1) * CHUNK)
                    kx = []
                    for k in range(KT):
                        t = kxp.tile([P, CHUNK], F32R, tag=f"k{k}", name=f"k{k}")
                        nc.sync.dma_start(out=t[:], in_=kxn_dram[k * P:(k + 1) * P, cs].bitcast(F32R))
                        kx.append(t)
                    stats_x = stp.tile([1, CHUNK], F32, tag="sx", name="sx")
                    stats_q = stp.tile([1, CHUNK], F32, tag="sq", name="sq")
                    for m in range(MT):
                        ps = aps.tile([P, CHUNK], F32, tag="ps", name="ps")
                        for k in range(KT):
                            nc.tensor.matmul(
                                ps[:],
                                lhsT=w_tiles[k][:, m * P:(m + 1) * P],
                                rhs=kx[k][:],
                                start=(k == 0), stop=(k == KT - 1))
                        vt = vp.tile([P, CHUNK], F32, tag="v", name="v")
                        nc.sync.dma_start(out=vt[:], in_=res_dram[m * P:(m + 1) * P, cs])
                        xt = out_tiles[m][:, cs]
                        nc.vector.tensor_add(xt, ps[:], vt[:])
                        nc.vector.tensor_scalar(
                            xt, xt, bias_pb[:, m:m + 1], None, OP.add)
                        sq = sqp.tile([P, CHUNK], F32R, tag="s", name="s")
                        nc.scalar.activation(sq[:], xt, AF.Square)
                        nc.tensor.matmul(stats_x[:], lhsT=ones_col[:],
                                         rhs=xt,
                                         start=(m == 0), stop=(m == MT - 1))
                        nc.tensor.matmul(stats_q[:], lhsT=ones_col[:],
                                         rhs=sq[:],
                                         start=(m == 0), stop=(m == MT - 1))
                    # column stats -> -mean, 1/std  ([1, CHUNK])
                    nm = sm.tile([1, CHUNK], F32, tag="nm", name="nm")
                    nc.scalar.activation(nm[:], stats_x[:], AF.Copy, scale=-1.0 / E)
                    t1 = sm.tile([1, CHUNK], F32, tag="t1", name="t1")
                    nc.scalar.activation(t1[:], stats_q[:], AF.Copy, scale=1.0 / E)
                    m2 = sm.tile([1, CHUNK], F32, tag="m2", name="m2")
                    nc.vector.tensor_mul(m2[:], nm[:], nm[:])
                    nc.vector.tensor_sub(t1[:], t1[:], m2[:])          # var
                    nc.scalar.activation(t1[:], t1[:], AF.Sqrt, bias=eps_t[:])
                    rs = sm.tile([1, CHUNK], F32, tag="rs", name="rs")
                    nc.vector.reciprocal(rs[:], t1[:])
                    # broadcast across partitions via K=1 matmuls (exact fp32)
                    nmB = bcp.tile([P, CHUNK], F32, tag="nmB", name="nmB")
                    nc.tensor.matmul(nmB[:], lhsT=ones_row[:], rhs=nm[:],
                                     start=True, stop=True)
                    rsB = bcp.tile([P, CHUNK], F32, tag="rsB", name="rsB")
                    nc.tensor.matmul(rsB[:], lhsT=ones_row[:], rhs=rs[:],
                                     start=True, stop=True)
                    for m in range(MT):
                        xt = out_tiles[m][:, cs]
                        nc.vector.tensor_add(xt, xt, nmB[:])
                        nc.vector.tensor_mul(xt, xt, rsB[:])
                        nc.vector.tensor_scalar(
                            xt, xt, g_pb[:, m:m + 1], b_pb[:, m:m + 1],
                            OP.mult, OP.add)
            return out_tiles

        def ffn_phase(sfx, in_tiles, h_tiles, w1_dram, b1_pb, w2_dram, b2_pb,
                      out_dram):
            """out = in + W2.T@gelu(W1.T@in + b1) + b2; streams to out_dram."""
            with ExitStack() as pctx:
                w1p = pctx.enter_context(tc.tile_pool(name=f"w1{sfx}", bufs=4))
                w2p = pctx.enter_context(tc.tile_pool(name=f"w2{sfx}", bufs=4))
                op = pctx.enter_context(tc.tile_pool(name=f"op{sfx}", bufs=2))
                fps = pctx.enter_context(
                    tc.tile_pool(name=f"fps{sfx}", bufs=2, space="PSUM"))
                for hm in range(HT):
                    ps = fps.tile([P, B], F32, tag="f", name="f")
                    for k in range(KT):
                        wt = w1p.tile([P, P], F32R, tag="w", name="w")
                        nc.sync.dma_start(
                            out=wt[:], in_=w1_dram[k * P:(k + 1) * P,
                                                   hm * P:(hm + 1) * P].bitcast(F32R))
                        for ns in range(B // 512):
                            nss = slice(ns * 512, (ns + 1) * 512)
                            nc.tensor.matmul(
                                ps[:, nss], lhsT=wt[:],
                                rhs=in_tiles[k][:, nss],
                                start=(k == 0), stop=(k == KT - 1))
                    nc.scalar.activation(h_tiles[hm][:], ps[:], AF.Gelu,
                                         bias=b1_pb[:, hm:hm + 1])
                for m in range(MT):
                    ps = fps.tile([P, B], F32, tag="f", name="f")
                    for k in range(HT):
                        wt = w2p.tile([P, P], BF16, tag="w", name="w")
                        nc.sync.dma_start(
                            out=wt[:], in_=w2_dram[k * P:(k + 1) * P,
                                                   m * P:(m + 1) * P])
                        for ns in range(B // 512):
                            nss = slice(ns * 512, (ns + 1) * 512)
                            nc.tensor.matmul(
                                ps[:, nss], lhsT=wt[:],
                                rhs=h_tiles[k][:, nss],
                                start=(k == 0), stop=(k == HT - 1))
                    ot = op.tile([P, B], F32, tag="o", name="o")
                    nc.vector.tensor_add(ot[:], ps[:], in_tiles[m][:])
                    nc.vector.tensor_scalar(
                        ot[:], ot[:], b2_pb[:, m:m + 1], None, OP.add)
                    nc.sync.dma_start(out=out_dram[m * P:(m + 1) * P, :], in_=ot[:])

        import os as _os
        _REP = int(_os.environ.get("BENCH_REPEAT", "1"))
        with ExitStack() as hctx:
            hp = hctx.enter_context(tc.tile_pool(name="hbf", bufs=1))

            def h_tiles():
                return [hp.tile([P, B], BF16, tag=f"h{i}", name=f"h{i}") for i in range(HT)]

            for _rep in range(_REP):
                # phase A: verb attends to noun, LN -> verb1 (resident)
                verb1 = attn_ln_phase(f"a{_rep}", nT, vT, wvo1, bvo1_pb,
                                      lnvg_pb, lnvb_pb)
                # phase B: verb FFN -> verb_out (DRAM)
                ffn_phase(f"b{_rep}", verb1, h_tiles(), w1v, b1v_pb, w2v,
                          b2v_pb, verb_out)
                # phase C: noun attends to verb2 (read back), LN -> noun1
                noun1 = attn_ln_phase(f"c{_rep}", verb_out, nT, wvo2, bvo2_pb,
                                      lnng_pb, lnnb_pb)
                # phase D: noun FFN -> noun_out
                ffn_phase(f"d{_rep}", noun1, h_tiles(), w1n, b1n_pb, w2n,
                          b2n_pb, noun_out)

    nc.finalize()
    return nc


_prog_cache = {}


def _get_program():
    if "nc" not in _prog_cache:
        _prog_cache["nc"] = _build_program()
    return _prog_cache["nc"]


def _prepare_maps(inputs):
    f32 = np.float32
    g = {k: np.asarray(v, f32) for k, v in inputs.items()}

    def fold(p):
        w = g[f"{p}_wo"] @ g[f"{p}_wv"]
        b = g[f"{p}_wo"] @ g[f"{p}_bv"] + g[f"{p}_bo"]
        return np.ascontiguousarray(w.T), np.ascontiguousarray(b)

    wvo1, bvo1 = fold("v2n")
    wvo2, bvo2 = fold("n2v")
    common = {
        "wvo1": wvo1, "bvo1": bvo1, "wvo2": wvo2, "bvo2": bvo2,
        "lnvg": g["ln_v_g"], "lnvb": g["ln_v_b"],
        "lnng": g["ln_n_g"], "lnnb": g["ln_n_b"],
        "w1v": np.ascontiguousarray(g["fv_w1"].T), "b1v": g["fv_b1"],
        "w2v": np.ascontiguousarray(g["fv_w2"].T).astype(ml_dtypes.bfloat16),
        "b2v": g["fv_b2"],
        "w1n": np.ascontiguousarray(g["fn_w1"].T), "b1n": g["fn_b1"],
        "w2n": np.ascontiguousarray(g["fn_w2"].T).astype(ml_dtypes.bfloat16),
        "b2n": g["fn_b2"],
        "ones_d": np.ones((128, 1), f32),
    }
    vT = np.ascontiguousarray(g["verb_features"].T)   # [E, 16384]
    nT = np.ascontiguousarray(g["noun_features"].T)
    in_maps = []
    for i in range(NCORES):
        cs = slice(i * B, (i + 1) * B)
        m = dict(common)
        m["vT"] = np.ascontiguousarray(vT[:, cs])
        m["nT"] = np.ascontiguousarray(nT[:, cs])
        in_maps.append(m)
    return in_maps


def kernel(**inputs):
    nc = _get_program()
    in_maps = _prepare_maps(inputs)
    res = run_bass_kernel_spmd(nc, in_maps, list(range(NCORES))).results
    verb = np.concatenate([res[i]["verb_out"] for i in range(NCORES)], axis=1)
    noun = np.concatenate([res[i]["noun_out"] for i in range(NCORES)], axis=1)
    return np.ascontiguousarray(verb.T), np.ascontiguousarray(noun.T)



# revision 13
# speedup vs baseline: 1.3255x; 1.3255x over previous
"""Trainium2 Bass kernel for a dense cross-task transformer block.

Math notes
----------
The reference "attention" has sequence length 1 on the key axis, so
softmax(scores) == 1.0 exactly and the whole q/k/score path is dead:

    mha_len1(q_in, kv_in, ...) == (kv_in @ wv.T + bv) @ wo.T + bo

which folds (on host) into a single matmul with W = wo @ wv and
b = wo @ bv + bo.  The block is then:

    verb1 = LN(verb + noun @ W1.T + c1)          (ln_v)
    verb2 = verb1 + FFN_v(verb1)
    noun1 = LN(noun + verb2 @ W2.T + c2)         (ln_n)
    noun2 = noun1 + FFN_n(noun1)
    return verb2, noun2

Device strategy
---------------
Pure data parallel over 8 cores (batch 16384 -> 2048 cols/core), weights
replicated.  Everything on device is feature-major ([E, batch]); every
matmul contracts along the SBUF partition dim.  The whole data path is
bf16 (fp32 PSUM accumulation), which halves HBM traffic vs f32 and
enables the fast-weight-load path on the PE.

LayerNorm stats use a replicated-reduction trick: the column sums
(sum_x, sum_x2) are computed with a [128,128] all-ones lhsT, so every
partition of the PSUM output already holds the per-column sum -- no
separate partition-broadcast matmuls are needed and the small-vector
math runs at full 128-lane width.

The residual stream stays SBUF-resident across all four phases (verb2
feeds phase C straight from SBUF; no DRAM round trip).  Weights live in
rep-scope pools whose tags are shared between phases A<->C and B<->D,
so phase C/D weight DMAs naturally begin while phase B/C compute runs
(prefetch via WAR dependencies).
"""

import numpy as np
import ml_dtypes
from contextlib import ExitStack

import concourse.bass as bass
import concourse.bacc as bacc_mod
import concourse.mybir as mybir
import concourse.tile as tile
from concourse.bass_utils import run_bass_kernel_spmd

E = 1024          # embed dim
H2 = 2048         # FFN hidden dim
B_TOTAL = 16384
NCORES = 8
B = B_TOTAL // NCORES   # 2048 cols per core
P = 128
EPS = 1e-5
C = 512           # column chunk
NCH = B // C      # 4
KT = E // P       # 8
MT = E // P       # 8
HT = H2 // P      # 16

F32 = mybir.dt.float32
F32R = mybir.dt.float32r
BF16 = mybir.dt.bfloat16
AF = mybir.ActivationFunctionType
OP = mybir.AluOpType

# packed per-partition vector table columns (f32 [128, NV])
_COL = {}
_ncol = 0
for _name, _n in (("bvo1", MT), ("bvo2", MT), ("lnvg", MT), ("lnvb", MT),
                  ("lnng", MT), ("lnnb", MT), ("b2v", MT), ("b2n", MT),
                  ("b1v", HT), ("b1n", HT)):
    _COL[_name] = _ncol
    _ncol += _n
NV = _ncol


def _build_program():
    nc = bacc_mod.Bacc("TRN2", target_bir_lowering=False)

    vT = nc.declare_dram_parameter("vT", [E, B], BF16, isOutput=False)
    nT = nc.declare_dram_parameter("nT", [E, B], BF16, isOutput=False)
    wvo1 = nc.declare_dram_parameter("wvo1", [E, E], BF16, isOutput=False)   # (wo@wv).T : [k, m]
    wvo2 = nc.declare_dram_parameter("wvo2", [E, E], BF16, isOutput=False)
    w1v = nc.declare_dram_parameter("w1v", [E, H2], BF16, isOutput=False)    # fv_w1.T
    w2v = nc.declare_dram_parameter("w2v", [H2, E], BF16, isOutput=False)    # fv_w2.T
    w1n = nc.declare_dram_parameter("w1n", [E, H2], BF16, isOutput=False)
    w2n = nc.declare_dram_parameter("w2n", [H2, E], BF16, isOutput=False)
    vecs = nc.declare_dram_parameter("vecs", [P, NV], F32, isOutput=False)
    ones_d = nc.declare_dram_parameter("ones_d", [P, P], F32, isOutput=False)
    verb_out = nc.declare_dram_parameter("verb_out", [E, B], F32, isOutput=True)
    noun_out = nc.declare_dram_parameter("noun_out", [E, B], F32, isOutput=True)

    with tile.TileContext(nc) as tc, ExitStack() as ctx:
        const = ctx.enter_context(tc.tile_pool(name="const", bufs=1))
        resid = ctx.enter_context(tc.tile_pool(name="resid", bufs=1))
        # rep-scope weight pools; tags shared A<->C (wvo) and B<->D (w1/w2)
        wvop = ctx.enter_context(tc.tile_pool(name="wvop", bufs=1))
        w1p = ctx.enter_context(tc.tile_pool(name="w1p", bufs=1))
        w2p = ctx.enter_context(tc.tile_pool(name="w2p", bufs=1))

        vec_t = const.tile([P, NV], F32, tag="vecs", name="vecs")
        nc.sync.dma_start(out=vec_t[:], in_=vecs[:, :])
        ones_t = const.tile([P, P], F32R, tag="ones", name="ones")
        nc.sync.dma_start(out=ones_t[:], in_=ones_d[:, :].bitcast(F32R))
        eps_t = const.tile([P, 1], F32, tag="eps", name="eps")
        nc.vector.memset(eps_t[:], EPS)

        def vcol(name, i):
            return vec_t[:, _COL[name] + i:_COL[name] + i + 1]

        # persistent residual-stream tiles ([128, B] bf16); verb1 in phases
        # A/B, overwritten as noun1 in phases C/D (same tags -> same slots)
        def a_tiles():
            return [resid.tile([P, B], BF16, tag=f"a{m}", name=f"a{m}") for m in range(MT)]

        def v2_tiles():
            return [resid.tile([P, B], BF16, tag=f"v{m}", name=f"v{m}") for m in range(MT)]

        def attn_ln_phase(sfx, w_dram, kx_dram, kx_sbuf, res_dram,
                          bias_name, g_name, b_name, out_tiles):
            """out[m] = LN(res + kx.T @ w + bias) feature-major, bf16."""
            with ExitStack() as pctx:
                kxp = pctx.enter_context(tc.tile_pool(name=f"kx{sfx}", bufs=2))
                rsp = pctx.enter_context(tc.tile_pool(name=f"rs{sfx}", bufs=1))
                xbp = pctx.enter_context(tc.tile_pool(name=f"xb{sfx}", bufs=1))
                sqp = pctx.enter_context(tc.tile_pool(name=f"sq{sfx}", bufs=2))
                smp = pctx.enter_context(tc.tile_pool(name=f"sm{sfx}", bufs=2))
                aps = pctx.enter_context(
                    tc.tile_pool(name=f"aps{sfx}", bufs=2, space="PSUM"))
                stp = pctx.enter_context(
                    tc.tile_pool(name=f"st{sfx}", bufs=2, space="PSUM"))

                # weight loads interleaved with chunk-0 rhs loads so the
                # first accumulation chain can start after ~2 tiles land
                w_tiles = []
                kx0 = []
                for k in range(KT):
                    wt = wvop.tile([P, E], BF16, tag=f"w{k}", name=f"w{k}")
                    nc.sync.dma_start(out=wt[:], in_=w_dram[k * P:(k + 1) * P, :])
                    w_tiles.append(wt)
                    if kx_sbuf is None:
                        t = kxp.tile([P, C], BF16, tag=f"k{k}", name=f"k{k}")
                        nc.sync.dma_start(out=t[:], in_=kx_dram[k * P:(k + 1) * P, 0:C])
                        kx0.append(t)

                for cc in range(NCH):
                    cs = slice(cc * C, (cc + 1) * C)
                    if kx_sbuf is None:
                        if cc == 0:
                            kx = kx0
                        else:
                            kx = []
                            for k in range(KT):
                                t = kxp.tile([P, C], BF16, tag=f"k{k}", name=f"k{k}")
                                nc.sync.dma_start(
                                    out=t[:], in_=kx_dram[k * P:(k + 1) * P, cs])
                                kx.append(t)
                        kx = [t[:] for t in kx]
                    else:
                        kx = [kx_sbuf[k][:, cs] for k in range(KT)]
                    res = []
                    for m in range(MT):
                        t = rsp.tile([P, C], BF16, tag=f"r{m}", name=f"r{m}")
                        nc.sync.dma_start(
                            out=t[:], in_=res_dram[m * P:(m + 1) * P, cs])
                        res.append(t)
                    sx_ps = stp.tile([P, C], F32, tag="sx", name="sx")
                    sq_ps = stp.tile([P, C], F32, tag="sq", name="sq")
                    xts = []
                    for m in range(MT):
                        ps = aps.tile([P, C], F32, tag="ps", name="ps")
                        for k in range(KT):
                            nc.tensor.matmul(
                                ps[:],
                                lhsT=w_tiles[k][:, m * P:(m + 1) * P],
                                rhs=kx[k],
                                start=(k == 0), stop=(k == KT - 1))
                        xt = xbp.tile([P, C], F32R, tag=f"x{m}", name=f"x{m}")
                        xts.append(xt)
                        # x = (psum + bias) + res   (f32: feeds LN stats)
                        nc.vector.scalar_tensor_tensor(
                            xt[:], ps[:], vcol(bias_name, m), res[m][:],
                            OP.add, OP.add)
                        sq = sqp.tile([P, C], F32R, tag="s", name="s")
                        nc.scalar.activation(sq[:], xt[:], AF.Square)
                        nc.tensor.matmul(sx_ps[:], lhsT=ones_t[:], rhs=xt[:],
                                         start=(m == 0), stop=(m == MT - 1))
                        nc.tensor.matmul(sq_ps[:], lhsT=ones_t[:], rhs=sq[:],
                                         start=(m == 0), stop=(m == MT - 1))
                    # replicated column stats -> -mean (bf16), 1/std (bf16)
                    nm_b = smp.tile([P, C], BF16, tag="nm", name="nm")
                    nc.scalar.activation(nm_b[:], sx_ps[:], AF.Copy,
                                         scale=-1.0 / E)
                    msq = smp.tile([P, C], F32, tag="msq", name="msq")
                    nc.scalar.activation(msq[:], sq_ps[:], AF.Copy,
                                         scale=1.0 / E)
                    mu2 = smp.tile([P, C], F32, tag="mu2", name="mu2")
                    nc.scalar.activation(mu2[:], nm_b[:], AF.Square)
                    nc.vector.tensor_sub(msq[:], msq[:], mu2[:])       # var
                    nc.scalar.activation(mu2[:], msq[:], AF.Sqrt, bias=eps_t[:])
                    nc.vector.reciprocal_approx_fast(msq[:], mu2[:])   # 1/std
                    rs_b = smp.tile([P, C], BF16, tag="rsb", name="rsb")
                    nc.scalar.activation(rs_b[:], msq[:], AF.Copy)
                    for m in range(MT):
                        t = smp.tile([P, C], BF16, tag="t", name="t")
                        nc.vector.tensor_add(t[:], xts[m][:], nm_b[:])
                        # out = (t * g) * rstd ; the LN beta is folded into
                        # the downstream FFN bias vectors on the host
                        nc.vector.scalar_tensor_tensor(
                            out_tiles[m][:, cs], t[:], vcol(g_name, m),
                            rs_b[:], OP.mult, OP.mult)

        def ffn_phase(sfx, in_tiles, w1_dram, b1_name, w2_dram, b2_name,
                      out_tiles, out_dram):
            """out = in + W2.T@gelu(W1.T@in + b1) + b2 -> out_tiles/out_dram."""
            with ExitStack() as pctx:
                hp = pctx.enter_context(tc.tile_pool(name=f"h{sfx}", bufs=2))
                stg = pctx.enter_context(tc.tile_pool(name=f"o{sfx}", bufs=1))
                hps = pctx.enter_context(
                    tc.tile_pool(name=f"hps{sfx}", bufs=2, space="PSUM"))
                ops = pctx.enter_context(
                    tc.tile_pool(name=f"ops{sfx}", bufs=2, space="PSUM"))

                # stream weights (rep-scope pools; tags shared with the
                # sibling FFN phase so next phase's loads prefetch early)
                w1t = {}
                for k in range(KT):
                    for hg in range(4):
                        t = w1p.tile([P, C], BF16, tag=f"w{k}_{hg}",
                                     name=f"w{k}_{hg}")
                        nc.sync.dma_start(
                            out=t[:], in_=w1_dram[k * P:(k + 1) * P,
                                                  hg * C:(hg + 1) * C])
                        w1t[(k, hg)] = t
                w2t = []
                for k in range(HT):
                    t = w2p.tile([P, E], BF16, tag=f"v{k}", name=f"v{k}")
                    nc.sync.dma_start(out=t[:], in_=w2_dram[k * P:(k + 1) * P, :])
                    w2t.append(t)

                for cc in range(NCH):
                    cs = slice(cc * C, (cc + 1) * C)
                    hts = []
                    for h in range(HT):
                        ps = hps.tile([P, C], F32, tag="f", name="f")
                        for k in range(KT):
                            nc.tensor.matmul(
                                ps[:],
                                lhsT=w1t[(k, h // 4)][:, (h % 4) * P:(h % 4 + 1) * P],
                                rhs=in_tiles[k][:, cs],
                                start=(k == 0), stop=(k == KT - 1))
                        ht = hp.tile([P, C], BF16, tag=f"h{h}", name=f"h{h}")
                        hts.append(ht)
                        nc.scalar.activation(ht[:], ps[:], AF.Gelu,
                                             bias=vcol(b1_name, h))
                    for m in range(MT):
                        ps = ops.tile([P, C], F32, tag="g", name="g")
                        for k in range(HT):
                            nc.tensor.matmul(
                                ps[:],
                                lhsT=w2t[k][:, m * P:(m + 1) * P],
                                rhs=hts[k][:],
                                start=(k == 0), stop=(k == HT - 1))
                        ot = stg.tile([P, C], F32, tag=f"s{m}", name=f"s{m}")
                        nc.vector.scalar_tensor_tensor(
                            ot[:], ps[:], vcol(b2_name, m), in_tiles[m][:, cs],
                            OP.add, OP.add)
                        nc.sync.dma_start(
                            out=out_dram[m * P:(m + 1) * P, cs], in_=ot[:])
                        if out_tiles is not None:
                            # bf16 copy kept resident as next phase's rhs
                            nc.scalar.activation(out_tiles[m][:, cs], ot[:],
                                                 AF.Copy)

        import os as _os
        _REP = int(_os.environ.get("BENCH_REPEAT", "1"))
        for _rep in range(_REP):
            # phase A: verb attends to noun, LN -> verb1 (resident)
            verb1 = a_tiles()
            attn_ln_phase(f"a{_rep}", wvo1, nT, None, vT,
                          "bvo1", "lnvg", "lnvb", verb1)
            # phase B: verb FFN -> verb2 (resident) + verb_out (DRAM)
            verb2 = v2_tiles()
            ffn_phase(f"b{_rep}", verb1, w1v, "b1v", w2v, "b2v",
                      verb2, verb_out)
            # phase C: noun attends to verb2 (SBUF), LN -> noun1
            noun1 = a_tiles()
            attn_ln_phase(f"c{_rep}", wvo2, None, verb2, nT,
                          "bvo2", "lnng", "lnnb", noun1)
            # phase D: noun FFN -> noun_out (DRAM only)
            ffn_phase(f"d{_rep}", noun1, w1n, "b1n", w2n, "b2n",
                      None, noun_out)

    nc.finalize()
    return nc


_prog_cache = {}


def _get_program():
    if "nc" not in _prog_cache:
        _prog_cache["nc"] = _build_program()
    return _prog_cache["nc"]


def _prepare_maps(inputs):
    f32 = np.float32
    bf16 = ml_dtypes.bfloat16
    g = {k: np.asarray(v, f32) for k, v in inputs.items()}

    def fold(p):
        w = g[f"{p}_wo"] @ g[f"{p}_wv"]
        b = g[f"{p}_wo"] @ g[f"{p}_bv"] + g[f"{p}_bo"]
        return np.ascontiguousarray(w.T).astype(bf16), np.ascontiguousarray(b)

    wvo1, bvo1 = fold("v2n")
    wvo2, bvo2 = fold("n2v")

    vec_tab = np.zeros((P, NV), f32)

    def pack(name, v):
        n = v.shape[0] // P
        vec_tab[:, _COL[name]:_COL[name] + n] = v.reshape(n, P).T

    pack("bvo1", bvo1)
    pack("bvo2", bvo2)
    pack("lnvg", g["ln_v_g"])
    pack("lnvb", g["ln_v_b"])
    pack("lnng", g["ln_n_g"])
    pack("lnnb", g["ln_n_b"])
    # LN beta folded into FFN biases: h = gelu(W1 @ (u + beta) + b1)
    # = gelu(W1 @ u + (b1 + W1 @ beta)); out = u + beta + d + b2
    pack("b2v", g["fv_b2"] + g["ln_v_b"])
    pack("b2n", g["fn_b2"] + g["ln_n_b"])
    pack("b1v", g["fv_b1"] + g["fv_w1"] @ g["ln_v_b"])
    pack("b1n", g["fn_b1"] + g["fn_w1"] @ g["ln_n_b"])

    common = {
        "wvo1": wvo1, "wvo2": wvo2,
        "w1v": np.ascontiguousarray(g["fv_w1"].T).astype(bf16),
        "w2v": np.ascontiguousarray(g["fv_w2"].T).astype(bf16),
        "w1n": np.ascontiguousarray(g["fn_w1"].T).astype(bf16),
        "w2n": np.ascontiguousarray(g["fn_w2"].T).astype(bf16),
        "vecs": vec_tab,
        "ones_d": np.ones((P, P), f32),
    }
    vTf = np.ascontiguousarray(g["verb_features"].T).astype(bf16)  # [E, 16384]
    nTf = np.ascontiguousarray(g["noun_features"].T).astype(bf16)
    in_maps = []
    for i in range(NCORES):
        cs = slice(i * B, (i + 1) * B)
        m = dict(common)
        m["vT"] = np.ascontiguousarray(vTf[:, cs])
        m["nT"] = np.ascontiguousarray(nTf[:, cs])
        in_maps.append(m)
    return in_maps


def kernel(**inputs):
    nc = _get_program()
    in_maps = _prepare_maps(inputs)
    res = run_bass_kernel_spmd(nc, in_maps, list(range(NCORES))).results
    verb = np.concatenate([res[i]["verb_out"] for i in range(NCORES)], axis=1)
    noun = np.concatenate([res[i]["noun_out"] for i in range(NCORES)], axis=1)
    return np.ascontiguousarray(verb.T), np.ascontiguousarray(noun.T)


# revision 15
# speedup vs baseline: 1.3509x; 1.0192x over previous
"""Trainium2 Bass kernel for a dense cross-task transformer block.

Math notes
----------
The reference "attention" has sequence length 1 on the key axis, so
softmax(scores) == 1.0 exactly and the whole q/k/score path is dead:

    mha_len1(q_in, kv_in, ...) == (kv_in @ wv.T + bv) @ wo.T + bo

which folds (on host) into a single matmul with W = wo @ wv and
b = wo @ bv + bo.  The block is then:

    verb1 = LN(verb + noun @ W1.T + c1)          (ln_v)
    verb2 = verb1 + FFN_v(verb1)
    noun1 = LN(noun + verb2 @ W2.T + c2)         (ln_n)
    noun2 = noun1 + FFN_n(noun1)
    return verb2, noun2

Device strategy
---------------
Pure data parallel over 8 cores (batch 16384 -> 2048 cols/core), weights
replicated.  Everything on device is feature-major ([E, batch]); every
matmul contracts along the SBUF partition dim.  The whole data path is
bf16 (fp32 PSUM accumulation), which halves HBM traffic vs f32 and
enables the fast-weight-load path on the PE.

LayerNorm stats use a replicated-reduction trick: the column sums
(sum_x, sum_x2) are computed with a [128,128] all-ones lhsT, so every
partition of the PSUM output already holds the per-column sum -- no
separate partition-broadcast matmuls are needed and the small-vector
math runs at full 128-lane width.

The residual stream stays SBUF-resident across all four phases (verb2
feeds phase C straight from SBUF; no DRAM round trip).  Weights live in
rep-scope pools whose tags are shared between phases A<->C and B<->D,
so phase C/D weight DMAs naturally begin while phase B/C compute runs
(prefetch via WAR dependencies).
"""

import numpy as np
import ml_dtypes
from contextlib import ExitStack

import concourse.bass as bass
import concourse.bacc as bacc_mod
import concourse.mybir as mybir
import concourse.tile as tile
from concourse.bass_utils import run_bass_kernel_spmd

E = 1024          # embed dim
H2 = 2048         # FFN hidden dim
B_TOTAL = 16384
NCORES = 8
B = B_TOTAL // NCORES   # 2048 cols per core
P = 128
EPS = 1e-5
C = 512           # column chunk
NCH = B // C      # 4
KT = E // P       # 8
MT = E // P       # 8
HT = H2 // P      # 16

F32 = mybir.dt.float32
F32R = mybir.dt.float32r
BF16 = mybir.dt.bfloat16
AF = mybir.ActivationFunctionType
OP = mybir.AluOpType

# packed per-partition vector table columns (f32 [128, NV])
_COL = {}
_ncol = 0
for _name, _n in (("bvo1", MT), ("bvo2", MT), ("lnvg", MT), ("lnvb", MT),
                  ("lnng", MT), ("lnnb", MT), ("b2v", MT), ("b2n", MT),
                  ("b1v", HT), ("b1n", HT)):
    _COL[_name] = _ncol
    _ncol += _n
NV = _ncol


def _build_program():
    nc = bacc_mod.Bacc("TRN2", target_bir_lowering=False)

    vT = nc.declare_dram_parameter("vT", [E, B], BF16, isOutput=False)
    nT = nc.declare_dram_parameter("nT", [E, B], BF16, isOutput=False)
    wvo1 = nc.declare_dram_parameter("wvo1", [E, E], BF16, isOutput=False)   # (wo@wv).T : [k, m]
    wvo2 = nc.declare_dram_parameter("wvo2", [E, E], BF16, isOutput=False)
    w1v = nc.declare_dram_parameter("w1v", [E, H2], BF16, isOutput=False)    # fv_w1.T
    w2v = nc.declare_dram_parameter("w2v", [H2, E], BF16, isOutput=False)    # fv_w2.T
    w1n = nc.declare_dram_parameter("w1n", [E, H2], BF16, isOutput=False)
    w2n = nc.declare_dram_parameter("w2n", [H2, E], BF16, isOutput=False)
    vecs = nc.declare_dram_parameter("vecs", [P, NV], F32, isOutput=False)
    ones_d = nc.declare_dram_parameter("ones_d", [P, P], F32, isOutput=False)
    verb_out = nc.declare_dram_parameter("verb_out", [E, B], F32, isOutput=True)
    noun_out = nc.declare_dram_parameter("noun_out", [E, B], F32, isOutput=True)

    with tile.TileContext(nc) as tc, ExitStack() as ctx:
        const = ctx.enter_context(tc.tile_pool(name="const", bufs=1))
        resid = ctx.enter_context(tc.tile_pool(name="resid", bufs=1))
        # rep-scope weight pools; tags shared A<->C (wvo) and B<->D (w1/w2)
        wvop = ctx.enter_context(tc.tile_pool(name="wvop", bufs=1))
        w1p = ctx.enter_context(tc.tile_pool(name="w1p", bufs=1))
        w2p = ctx.enter_context(tc.tile_pool(name="w2p", bufs=1))

        vec_t = const.tile([P, NV], F32, tag="vecs", name="vecs")
        nc.sync.dma_start(out=vec_t[:], in_=vecs[:, :])
        ones_t = const.tile([P, P], F32R, tag="ones", name="ones")
        nc.sync.dma_start(out=ones_t[:], in_=ones_d[:, :].bitcast(F32R))
        ones_b = const.tile([P, P], BF16, tag="onesb", name="onesb")
        nc.vector.memset(ones_b[:], 1.0)
        eps_t = const.tile([P, 1], F32, tag="eps", name="eps")
        nc.vector.memset(eps_t[:], EPS)

        def vcol(name, i):
            return vec_t[:, _COL[name] + i:_COL[name] + i + 1]

        # persistent residual-stream tiles ([128, B] bf16); verb1 in phases
        # A/B, overwritten as noun1 in phases C/D (same tags -> same slots)
        def a_tiles():
            return [resid.tile([P, B], BF16, tag=f"a{m}", name=f"a{m}") for m in range(MT)]

        def v2_tiles():
            return [resid.tile([P, B], BF16, tag=f"v{m}", name=f"v{m}") for m in range(MT)]

        def attn_ln_phase(sfx, w_dram, kx_dram, kx_sbuf, res_dram,
                          bias_name, g_name, b_name, out_tiles):
            """out[m] = LN(res + kx.T @ w + bias) feature-major, bf16."""
            with ExitStack() as pctx:
                kxp = pctx.enter_context(tc.tile_pool(name=f"kx{sfx}", bufs=2))
                rsp = pctx.enter_context(tc.tile_pool(name=f"rs{sfx}", bufs=1))
                xbp = pctx.enter_context(tc.tile_pool(name=f"xb{sfx}", bufs=1))
                sqp = pctx.enter_context(tc.tile_pool(name=f"sq{sfx}", bufs=1))
                smp = pctx.enter_context(tc.tile_pool(name=f"sm{sfx}", bufs=2))
                aps = pctx.enter_context(
                    tc.tile_pool(name=f"aps{sfx}", bufs=2, space="PSUM"))
                stp = pctx.enter_context(
                    tc.tile_pool(name=f"st{sfx}", bufs=2, space="PSUM"))

                # weight loads (own DMA queue) interleaved with chunk-0
                # rhs loads so the first accumulation chain starts early
                w_tiles = []
                def load_kx(cc):
                    cs0 = slice(cc * C, (cc + 1) * C)
                    out = []
                    for k in range(KT):
                        t = kxp.tile([P, C], BF16, tag=f"k{k}", name=f"k{k}")
                        nc.sync.dma_start(
                            out=t[:], in_=kx_dram[k * P:(k + 1) * P, cs0])
                        out.append(t)
                    return out

                for k in range(KT):
                    wt = wvop.tile([P, E], BF16, tag=f"w{k}", name=f"w{k}")
                    nc.gpsimd.dma_start(out=wt[:], in_=w_dram[k * P:(k + 1) * P, :])
                    w_tiles.append(wt)
                kx_next = load_kx(0) if kx_sbuf is None else None

                for cc in range(NCH):
                    cs = slice(cc * C, (cc + 1) * C)
                    if kx_sbuf is None:
                        kx_cur = kx_next
                        if cc + 1 < NCH:
                            kx_next = load_kx(cc + 1)
                        kx = [t[:] for t in kx_cur]
                    else:
                        kx = [kx_sbuf[k][:, cs] for k in range(KT)]
                    res = []
                    for m in range(MT):
                        t = rsp.tile([P, C], BF16, tag=f"r{m}", name=f"r{m}")
                        nc.sync.dma_start(
                            out=t[:], in_=res_dram[m * P:(m + 1) * P, cs])
                        res.append(t)
                    sx_ps = stp.tile([P, C], F32, tag="sx", name="sx")
                    sq_ps = stp.tile([P, C], F32, tag="sq", name="sq")
                    xts = []
                    sqs = []
                    for m in range(MT):
                        ps = aps.tile([P, C], F32, tag="ps", name="ps")
                        for k in range(KT):
                            nc.tensor.matmul(
                                ps[:],
                                lhsT=w_tiles[k][:, m * P:(m + 1) * P],
                                rhs=kx[k],
                                start=(k == 0), stop=(k == KT - 1))
                        xt = xbp.tile([P, C], F32R, tag=f"x{m}", name=f"x{m}")
                        xts.append(xt)
                        # x = (psum + bias) + res   (f32: feeds LN stats)
                        nc.vector.scalar_tensor_tensor(
                            xt[:], ps[:], vcol(bias_name, m), res[m][:],
                            OP.add, OP.add)
                        sq = sqp.tile([P, C], BF16, tag=f"s{m}", name=f"s{m}")
                        sqs.append(sq)
                        nc.scalar.activation(sq[:], xt[:], AF.Square)
                    # stats batched after the mains: PE never waits on the
                    # x/sq eviction chain mid-chunk
                    for m in range(MT):
                        nc.tensor.matmul(sx_ps[:], lhsT=ones_t[:], rhs=xts[m][:],
                                         start=(m == 0), stop=(m == MT - 1))
                    for m in range(MT):
                        nc.tensor.matmul(sq_ps[:], lhsT=ones_b[:], rhs=sqs[m][:],
                                         start=(m == 0), stop=(m == MT - 1))
                    # replicated column stats -> -mean (bf16), 1/std (bf16)
                    nm_b = smp.tile([P, C], BF16, tag="nm", name="nm")
                    nc.scalar.activation(nm_b[:], sx_ps[:], AF.Copy,
                                         scale=-1.0 / E)
                    msq = smp.tile([P, C], F32, tag="msq", name="msq")
                    nc.scalar.activation(msq[:], sq_ps[:], AF.Copy,
                                         scale=1.0 / E)
                    mu2 = smp.tile([P, C], F32, tag="mu2", name="mu2")
                    nc.scalar.activation(mu2[:], nm_b[:], AF.Square)
                    nc.vector.tensor_sub(msq[:], msq[:], mu2[:])       # var
                    nc.scalar.activation(mu2[:], msq[:], AF.Sqrt, bias=eps_t[:])
                    nc.vector.reciprocal_approx_fast(msq[:], mu2[:])   # 1/std
                    rs_b = smp.tile([P, C], BF16, tag="rsb", name="rsb")
                    nc.scalar.activation(rs_b[:], msq[:], AF.Copy)
                    for m in range(MT):
                        t = smp.tile([P, C], BF16, tag="t", name="t")
                        nc.vector.tensor_add(t[:], xts[m][:], nm_b[:])
                        # out = (t * g) * rstd ; the LN beta is folded into
                        # the downstream FFN bias vectors on the host
                        nc.vector.scalar_tensor_tensor(
                            out_tiles[m][:, cs], t[:], vcol(g_name, m),
                            rs_b[:], OP.mult, OP.mult)

        def ffn_phase(sfx, in_tiles, w1_dram, b1_name, w2_dram, b2_name,
                      out_tiles, out_dram):
            """out = in + W2.T@gelu(W1.T@in + b1) + b2 -> out_tiles/out_dram."""
            with ExitStack() as pctx:
                hp = pctx.enter_context(tc.tile_pool(name=f"h{sfx}", bufs=2))
                stg = pctx.enter_context(tc.tile_pool(name=f"o{sfx}", bufs=1))
                hps = pctx.enter_context(
                    tc.tile_pool(name=f"hps{sfx}", bufs=2, space="PSUM"))
                ops = pctx.enter_context(
                    tc.tile_pool(name=f"ops{sfx}", bufs=2, space="PSUM"))

                # stream weights (rep-scope pools; tags shared with the
                # sibling FFN phase so next phase's loads prefetch early)
                w1t = {}
                for k in range(KT):
                    for hg in range(4):
                        t = w1p.tile([P, C], BF16, tag=f"w{k}_{hg}",
                                     name=f"w{k}_{hg}")
                        nc.gpsimd.dma_start(
                            out=t[:], in_=w1_dram[k * P:(k + 1) * P,
                                                  hg * C:(hg + 1) * C])
                        w1t[(k, hg)] = t
                w2t = []
                for k in range(HT):
                    t = w2p.tile([P, E], BF16, tag=f"v{k}", name=f"v{k}")
                    nc.gpsimd.dma_start(out=t[:], in_=w2_dram[k * P:(k + 1) * P, :])
                    w2t.append(t)

                for cc in range(NCH):
                    cs = slice(cc * C, (cc + 1) * C)
                    hts = []
                    for h in range(HT):
                        ps = hps.tile([P, C], F32, tag="f", name="f")
                        for k in range(KT):
                            nc.tensor.matmul(
                                ps[:],
                                lhsT=w1t[(k, h // 4)][:, (h % 4) * P:(h % 4 + 1) * P],
                                rhs=in_tiles[k][:, cs],
                                start=(k == 0), stop=(k == KT - 1))
                        ht = hp.tile([P, C], BF16, tag=f"h{h}", name=f"h{h}")
                        hts.append(ht)
                        nc.scalar.activation(ht[:], ps[:], AF.Gelu,
                                             bias=vcol(b1_name, h))
                    for m in range(MT):
                        ps = ops.tile([P, C], F32, tag="g", name="g")
                        for k in range(HT):
                            nc.tensor.matmul(
                                ps[:],
                                lhsT=w2t[k][:, m * P:(m + 1) * P],
                                rhs=hts[k][:],
                                start=(k == 0), stop=(k == HT - 1))
                        ot = stg.tile([P, C], F32, tag=f"s{m}", name=f"s{m}")
                        nc.vector.scalar_tensor_tensor(
                            ot[:], ps[:], vcol(b2_name, m), in_tiles[m][:, cs],
                            OP.add, OP.add)
                        nc.scalar.dma_start(
                            out=out_dram[m * P:(m + 1) * P, cs], in_=ot[:])
                        if out_tiles is not None:
                            # bf16 copy kept resident as next phase's rhs
                            nc.scalar.activation(out_tiles[m][:, cs], ot[:],
                                                 AF.Copy)

        import os as _os
        _REP = int(_os.environ.get("BENCH_REPEAT", "1"))
        for _rep in range(_REP):
            # phase A: verb attends to noun, LN -> verb1 (resident)
            verb1 = a_tiles()
            attn_ln_phase(f"a{_rep}", wvo1, nT, None, vT,
                          "bvo1", "lnvg", "lnvb", verb1)
            # phase B: verb FFN -> verb2 (resident) + verb_out (DRAM)
            verb2 = v2_tiles()
            ffn_phase(f"b{_rep}", verb1, w1v, "b1v", w2v, "b2v",
                      verb2, verb_out)
            # phase C: noun attends to verb2 (SBUF), LN -> noun1
            noun1 = a_tiles()
            attn_ln_phase(f"c{_rep}", wvo2, None, verb2, nT,
                          "bvo2", "lnng", "lnnb", noun1)
            # phase D: noun FFN -> noun_out (DRAM only)
            ffn_phase(f"d{_rep}", noun1, w1n, "b1n", w2n, "b2n",
                      None, noun_out)

    nc.finalize()
    return nc


_prog_cache = {}


def _get_program():
    if "nc" not in _prog_cache:
        _prog_cache["nc"] = _build_program()
    return _prog_cache["nc"]


def _prepare_maps(inputs):
    f32 = np.float32
    bf16 = ml_dtypes.bfloat16
    g = {k: np.asarray(v, f32) for k, v in inputs.items()}

    def fold(p):
        w = g[f"{p}_wo"] @ g[f"{p}_wv"]
        b = g[f"{p}_wo"] @ g[f"{p}_bv"] + g[f"{p}_bo"]
        return np.ascontiguousarray(w.T).astype(bf16), np.ascontiguousarray(b)

    wvo1, bvo1 = fold("v2n")
    wvo2, bvo2 = fold("n2v")

    vec_tab = np.zeros((P, NV), f32)

    def pack(name, v):
        n = v.shape[0] // P
        vec_tab[:, _COL[name]:_COL[name] + n] = v.reshape(n, P).T

    pack("bvo1", bvo1)
    pack("bvo2", bvo2)
    pack("lnvg", g["ln_v_g"])
    pack("lnvb", g["ln_v_b"])
    pack("lnng", g["ln_n_g"])
    pack("lnnb", g["ln_n_b"])
    # LN beta folded into FFN biases: h = gelu(W1 @ (u + beta) + b1)
    # = gelu(W1 @ u + (b1 + W1 @ beta)); out = u + beta + d + b2
    pack("b2v", g["fv_b2"] + g["ln_v_b"])
    pack("b2n", g["fn_b2"] + g["ln_n_b"])
    pack("b1v", g["fv_b1"] + g["fv_w1"] @ g["ln_v_b"])
    pack("b1n", g["fn_b1"] + g["fn_w1"] @ g["ln_n_b"])

    common = {
        "wvo1": wvo1, "wvo2": wvo2,
        "w1v": np.ascontiguousarray(g["fv_w1"].T).astype(bf16),
        "w2v": np.ascontiguousarray(g["fv_w2"].T).astype(bf16),
        "w1n": np.ascontiguousarray(g["fn_w1"].T).astype(bf16),
        "w2n": np.ascontiguousarray(g["fn_w2"].T).astype(bf16),
        "vecs": vec_tab,
        "ones_d": np.ones((P, P), f32),
    }
    vTf = np.ascontiguousarray(g["verb_features"].T).astype(bf16)  # [E, 16384]
    nTf = np.ascontiguousarray(g["noun_features"].T).astype(bf16)
    in_maps = []
    for i in range(NCORES):
        cs = slice(i * B, (i + 1) * B)
        m = dict(common)
        m["vT"] = np.ascontiguousarray(vTf[:, cs])
        m["nT"] = np.ascontiguousarray(nTf[:, cs])
        in_maps.append(m)
    return in_maps


def kernel(**inputs):
    nc = _get_program()
    in_maps = _prepare_maps(inputs)
    res = run_bass_kernel_spmd(nc, in_maps, list(range(NCORES))).results
    verb = np.concatenate([res[i]["verb_out"] for i in range(NCORES)], axis=1)
    noun = np.concatenate([res[i]["noun_out"] for i in range(NCORES)], axis=1)
    return np.ascontiguousarray(verb.T), np.ascontiguousarray(noun.T)


# revision 20
# speedup vs baseline: 1.3829x; 1.0237x over previous
"""Trainium2 Bass kernel for a dense cross-task transformer block.

Math notes
----------
The reference "attention" has sequence length 1 on the key axis, so
softmax(scores) == 1.0 exactly and the whole q/k/score path is dead:

    mha_len1(q_in, kv_in, ...) == (kv_in @ wv.T + bv) @ wo.T + bo

which folds (on host) into a single matmul with W = wo @ wv and
b = wo @ bv + bo.  The block is then:

    verb1 = LN(verb + noun @ W1.T + c1)          (ln_v)
    verb2 = verb1 + FFN_v(verb1)
    noun1 = LN(noun + verb2 @ W2.T + c2)         (ln_n)
    noun2 = noun1 + FFN_n(noun1)
    return verb2, noun2

Device strategy
---------------
Pure data parallel over 8 cores (batch 16384 -> 2048 cols/core), weights
replicated.  Everything on device is feature-major ([E, batch]); every
matmul contracts along the SBUF partition dim.  The whole data path is
bf16 (fp32 PSUM accumulation), which halves HBM traffic vs f32 and
enables the fast-weight-load path on the PE.

LayerNorm stats use a replicated-reduction trick: the column sums
(sum_x, sum_x2) are computed with a [128,128] all-ones lhsT, so every
partition of the PSUM output already holds the per-column sum -- no
separate partition-broadcast matmuls are needed and the small-vector
math runs at full 128-lane width.

The residual stream stays SBUF-resident across all four phases (verb2
feeds phase C straight from SBUF; no DRAM round trip).  Weights live in
rep-scope pools whose tags are shared between phases A<->C and B<->D,
so phase C/D weight DMAs naturally begin while phase B/C compute runs
(prefetch via WAR dependencies).
"""

import numpy as np
import ml_dtypes
from contextlib import ExitStack

import concourse.bass as bass
import concourse.bacc as bacc_mod
import concourse.mybir as mybir
import concourse.tile as tile
from concourse.bass_utils import run_bass_kernel_spmd

E = 1024          # embed dim
H2 = 2048         # FFN hidden dim
B_TOTAL = 16384
NCORES = 8
B = B_TOTAL // NCORES   # 2048 cols per core
P = 128
EPS = 1e-5
C = 512           # column chunk
NCH = B // C      # 4
KT = E // P       # 8
MT = E // P       # 8
HT = H2 // P      # 16

F32 = mybir.dt.float32
F32R = mybir.dt.float32r
BF16 = mybir.dt.bfloat16
AF = mybir.ActivationFunctionType
OP = mybir.AluOpType

# packed per-partition vector table columns (f32 [128, NV])
_COL = {}
_ncol = 0
for _name, _n in (("bvo1", MT), ("bvo2", MT), ("lnvg", MT), ("lnvb", MT),
                  ("lnng", MT), ("lnnb", MT), ("b2v", MT), ("b2n", MT),
                  ("b1v", HT), ("b1n", HT)):
    _COL[_name] = _ncol
    _ncol += _n
NV = _ncol


def _build_program():
    nc = bacc_mod.Bacc("TRN2", target_bir_lowering=False)

    vT = nc.declare_dram_parameter("vT", [E, B], BF16, isOutput=False)
    nT = nc.declare_dram_parameter("nT", [E, B], BF16, isOutput=False)
    wvo1 = nc.declare_dram_parameter("wvo1", [E, E], BF16, isOutput=False)   # (wo@wv).T : [k, m]
    wvo2 = nc.declare_dram_parameter("wvo2", [E, E], BF16, isOutput=False)
    w1v = nc.declare_dram_parameter("w1v", [E, H2], BF16, isOutput=False)    # fv_w1.T
    w2v = nc.declare_dram_parameter("w2v", [H2, E], BF16, isOutput=False)    # fv_w2.T
    w1n = nc.declare_dram_parameter("w1n", [E, H2], BF16, isOutput=False)
    w2n = nc.declare_dram_parameter("w2n", [H2, E], BF16, isOutput=False)
    vecs = nc.declare_dram_parameter("vecs", [P, NV], F32, isOutput=False)
    ones_d = nc.declare_dram_parameter("ones_d", [P, P], F32, isOutput=False)
    verb_out = nc.declare_dram_parameter("verb_out", [E, B], F32, isOutput=True)
    noun_out = nc.declare_dram_parameter("noun_out", [E, B], F32, isOutput=True)

    with tile.TileContext(nc) as tc, ExitStack() as ctx:
        const = ctx.enter_context(tc.tile_pool(name="const", bufs=1))
        resid = ctx.enter_context(tc.tile_pool(name="resid", bufs=1))
        # rep-scope weight pools; tags shared A<->C (wvo) and B<->D (w1/w2)
        wvop = ctx.enter_context(tc.tile_pool(name="wvop", bufs=1))
        w1p = ctx.enter_context(tc.tile_pool(name="w1p", bufs=1))
        w2p = ctx.enter_context(tc.tile_pool(name="w2p", bufs=1))

        vec_t = const.tile([P, NV], F32, tag="vecs", name="vecs")
        nc.sync.dma_start(out=vec_t[:], in_=vecs[:, :])
        ones_t = const.tile([P, P], F32R, tag="ones", name="ones")
        nc.sync.dma_start(out=ones_t[:], in_=ones_d[:, :].bitcast(F32R))
        ones_b = const.tile([P, P], BF16, tag="onesb", name="onesb")
        nc.vector.memset(ones_b[:], 1.0)
        eps_t = const.tile([P, 1], F32, tag="eps", name="eps")
        nc.vector.memset(eps_t[:], EPS)

        def vcol(name, i):
            return vec_t[:, _COL[name] + i:_COL[name] + i + 1]

        # persistent residual-stream tiles ([128, B] bf16); verb1 in phases
        # A/B, overwritten as noun1 in phases C/D (same tags -> same slots)
        def a_tiles():
            return [resid.tile([P, B], BF16, tag=f"a{m}", name=f"a{m}") for m in range(MT)]

        def v2_tiles():
            return [resid.tile([P, B], BF16, tag=f"v{m}", name=f"v{m}") for m in range(MT)]

        def attn_ln_phase(sfx, w_dram, kx_dram, kx_sbuf, res_dram,
                          bias_name, g_name, b_name, out_tiles):
            """out[m] = LN(res + kx.T @ w + bias) feature-major, bf16."""
            with ExitStack() as pctx:
                kxp = pctx.enter_context(tc.tile_pool(name=f"kx{sfx}", bufs=2))
                rsp = pctx.enter_context(tc.tile_pool(name=f"rs{sfx}", bufs=1))
                xbp = pctx.enter_context(tc.tile_pool(name=f"xb{sfx}", bufs=1))
                sqp = pctx.enter_context(tc.tile_pool(name=f"sq{sfx}", bufs=1))
                smp = pctx.enter_context(tc.tile_pool(name=f"sm{sfx}", bufs=2))
                aps = pctx.enter_context(
                    tc.tile_pool(name=f"aps{sfx}", bufs=2, space="PSUM"))
                stp = pctx.enter_context(
                    tc.tile_pool(name=f"st{sfx}", bufs=2, space="PSUM"))

                # weight loads (own DMA queue) interleaved with chunk-0
                # rhs loads so the first accumulation chain starts early
                w_tiles = []
                def load_kx(cc):
                    cs0 = slice(cc * C, (cc + 1) * C)
                    out = []
                    for k in range(KT):
                        t = kxp.tile([P, C], BF16, tag=f"k{k}", name=f"k{k}")
                        nc.sync.dma_start(
                            out=t[:], in_=kx_dram[k * P:(k + 1) * P, cs0])
                        out.append(t)
                    return out

                for k in range(KT):
                    wt = wvop.tile([P, E], BF16, tag=f"w{k}", name=f"w{k}")
                    nc.scalar.dma_start(out=wt[:], in_=w_dram[k * P:(k + 1) * P, :])
                    w_tiles.append(wt)
                kx_next = load_kx(0) if kx_sbuf is None else None

                def apply_pre(prev, m):
                    # gpsimd mean-subtract for the PREVIOUS chunk's tile m
                    # (must precede this chunk's pass1 overwrite of x[m])
                    pxts, pcs, pnm, pmsq = prev
                    t = smp.tile([P, C], BF16, tag="t", name="t")
                    nc.gpsimd.tensor_add(t[:], pxts[m][:].bitcast(F32),
                                         pnm[:])
                    return t

                def apply_post(prev, m, t):
                    # out = (t * g) * rstd ; LN beta folded into FFN biases
                    pxts, pcs, pnm, pmsq = prev
                    nc.vector.scalar_tensor_tensor(
                        out_tiles[m][:, pcs], t[:], vcol(g_name, m),
                        pmsq[:], OP.mult, OP.mult)

                prev = None
                for cc in range(NCH):
                    cs = slice(cc * C, (cc + 1) * C)
                    if kx_sbuf is None:
                        kx_cur = kx_next
                        if cc + 1 < NCH:
                            kx_next = load_kx(cc + 1)
                        kx = [t[:] for t in kx_cur]
                    else:
                        kx = [kx_sbuf[k][:, cs] for k in range(KT)]
                    res = []
                    for m in range(MT):
                        t = rsp.tile([P, C], BF16, tag=f"r{m}", name=f"r{m}")
                        nc.sync.dma_start(
                            out=t[:], in_=res_dram[m * P:(m + 1) * P, cs])
                        res.append(t)
                    sx_ps = stp.tile([P, C], F32, tag="sx", name="sx")
                    sq_ps = stp.tile([P, C], F32, tag="sq", name="sq")
                    xts = []
                    sqs = []
                    for m in range(MT):
                        if prev is not None:
                            tprev = apply_pre(prev, m)
                        ps = aps.tile([P, C], F32, tag="ps", name="ps")
                        for k in range(KT):
                            nc.tensor.matmul(
                                ps[:],
                                lhsT=w_tiles[k][:, m * P:(m + 1) * P],
                                rhs=kx[k],
                                start=(k == 0), stop=(k == KT - 1))
                        xt = xbp.tile([P, C], F32R, tag=f"x{m}", name=f"x{m}")
                        xts.append(xt)
                        # x = (psum + bias) + res   (f32: feeds LN stats)
                        nc.vector.scalar_tensor_tensor(
                            xt[:], ps[:], vcol(bias_name, m), res[m][:],
                            OP.add, OP.add)
                        sq = sqp.tile([P, C], BF16, tag=f"s{m}", name=f"s{m}")
                        sqs.append(sq)
                        nc.scalar.activation(sq[:], xt[:], AF.Square)
                        if prev is not None:
                            apply_post(prev, m, tprev)
                    # stats batched after the mains: PE never waits on the
                    # x/sq eviction chain mid-chunk
                    for m in range(MT):
                        nc.tensor.matmul(sx_ps[:], lhsT=ones_t[:], rhs=xts[m][:],
                                         start=(m == 0), stop=(m == MT - 1))
                    for m in range(MT):
                        nc.tensor.matmul(sq_ps[:], lhsT=ones_b[:], rhs=sqs[m][:],
                                         start=(m == 0), stop=(m == MT - 1))
                    # replicated column stats -> -mean, 1/std
                    nm_b = smp.tile([P, C], F32, tag="nm", name="nm")
                    nc.scalar.activation(nm_b[:], sx_ps[:], AF.Copy,
                                         scale=-1.0 / E)
                    msq = smp.tile([P, C], F32, tag="msq", name="msq")
                    nc.scalar.activation(msq[:], sq_ps[:], AF.Copy,
                                         scale=1.0 / E)
                    mu2 = smp.tile([P, C], F32, tag="mu2", name="mu2")
                    nc.scalar.activation(mu2[:], nm_b[:], AF.Square)
                    nc.gpsimd.tensor_sub(msq[:], msq[:], mu2[:])       # var
                    nc.scalar.activation(mu2[:], msq[:], AF.Sqrt, bias=eps_t[:])
                    nc.vector.reciprocal_approx_fast(msq[:], mu2[:])   # 1/std
                    prev = (xts, cs, nm_b, msq)
                # drain: apply for the final chunk
                for m in range(MT):
                    t = apply_pre(prev, m)
                    apply_post(prev, m, t)

        def ffn_phase(sfx, in_tiles, w1_dram, b1_name, w2_dram, b2_name,
                      out_tiles, out_dram):
            """out = in + W2.T@gelu(W1.T@in + b1) + b2 -> out_tiles/out_dram."""
            with ExitStack() as pctx:
                hp = pctx.enter_context(tc.tile_pool(name=f"h{sfx}", bufs=2))
                stg = pctx.enter_context(tc.tile_pool(name=f"o{sfx}", bufs=1))
                hps = pctx.enter_context(
                    tc.tile_pool(name=f"hps{sfx}", bufs=2, space="PSUM"))
                ops = pctx.enter_context(
                    tc.tile_pool(name=f"ops{sfx}", bufs=2, space="PSUM"))

                # stream weights (rep-scope pools; tags shared with the
                # sibling FFN phase so next phase's loads prefetch early)
                w1t = {}
                for k in range(KT):
                    for hg in range(4):
                        t = w1p.tile([P, C], BF16, tag=f"w{k}_{hg}",
                                     name=f"w{k}_{hg}")
                        nc.scalar.dma_start(
                            out=t[:], in_=w1_dram[k * P:(k + 1) * P,
                                                  hg * C:(hg + 1) * C])
                        w1t[(k, hg)] = t
                w2t = []
                for k in range(HT):
                    t = w2p.tile([P, E], BF16, tag=f"v{k}", name=f"v{k}")
                    nc.scalar.dma_start(out=t[:], in_=w2_dram[k * P:(k + 1) * P, :])
                    w2t.append(t)

                for cc in range(NCH):
                    cs = slice(cc * C, (cc + 1) * C)
                    hts = []
                    for h in range(HT):
                        ps = hps.tile([P, C], F32, tag="f", name="f")
                        for k in range(KT):
                            nc.tensor.matmul(
                                ps[:],
                                lhsT=w1t[(k, h // 4)][:, (h % 4) * P:(h % 4 + 1) * P],
                                rhs=in_tiles[k][:, cs],
                                start=(k == 0), stop=(k == KT - 1))
                        ht = hp.tile([P, C], BF16, tag=f"h{h}", name=f"h{h}")
                        hts.append(ht)
                        nc.scalar.activation(ht[:], ps[:], AF.Gelu,
                                             bias=vcol(b1_name, h))
                    for m in range(MT):
                        ps = ops.tile([P, C], F32, tag="g", name="g")
                        for k in range(HT):
                            nc.tensor.matmul(
                                ps[:],
                                lhsT=w2t[k][:, m * P:(m + 1) * P],
                                rhs=hts[k][:],
                                start=(k == 0), stop=(k == HT - 1))
                        ot = stg.tile([P, C], F32, tag=f"s{m}", name=f"s{m}")
                        nc.vector.scalar_tensor_tensor(
                            ot[:], ps[:], vcol(b2_name, m), in_tiles[m][:, cs],
                            OP.add, OP.add)
                        nc.scalar.dma_start(
                            out=out_dram[m * P:(m + 1) * P, cs], in_=ot[:])
                        if out_tiles is not None:
                            # bf16 copy kept resident as next phase's rhs
                            nc.scalar.activation(out_tiles[m][:, cs], ot[:],
                                                 AF.Copy)

        import os as _os
        _REP = int(_os.environ.get("BENCH_REPEAT", "1"))
        for _rep in range(_REP):
            # phase A: verb attends to noun, LN -> verb1 (resident)
            verb1 = a_tiles()
            attn_ln_phase(f"a{_rep}", wvo1, nT, None, vT,
                          "bvo1", "lnvg", "lnvb", verb1)
            # phase B: verb FFN -> verb2 (resident) + verb_out (DRAM)
            verb2 = v2_tiles()
            ffn_phase(f"b{_rep}", verb1, w1v, "b1v", w2v, "b2v",
                      verb2, verb_out)
            # phase C: noun attends to verb2 (SBUF), LN -> noun1
            noun1 = a_tiles()
            attn_ln_phase(f"c{_rep}", wvo2, None, verb2, nT,
                          "bvo2", "lnng", "lnnb", noun1)
            # phase D: noun FFN -> noun_out (DRAM only)
            ffn_phase(f"d{_rep}", noun1, w1n, "b1n", w2n, "b2n",
                      None, noun_out)

    nc.finalize()
    return nc


_prog_cache = {}


def _get_program():
    if "nc" not in _prog_cache:
        _prog_cache["nc"] = _build_program()
    return _prog_cache["nc"]


def _prepare_maps(inputs):
    f32 = np.float32
    bf16 = ml_dtypes.bfloat16
    g = {k: np.asarray(v, f32) for k, v in inputs.items()}

    def fold(p):
        w = g[f"{p}_wo"] @ g[f"{p}_wv"]
        b = g[f"{p}_wo"] @ g[f"{p}_bv"] + g[f"{p}_bo"]
        return np.ascontiguousarray(w.T).astype(bf16), np.ascontiguousarray(b)

    wvo1, bvo1 = fold("v2n")
    wvo2, bvo2 = fold("n2v")

    vec_tab = np.zeros((P, NV), f32)

    def pack(name, v):
        n = v.shape[0] // P
        vec_tab[:, _COL[name]:_COL[name] + n] = v.reshape(n, P).T

    pack("bvo1", bvo1)
    pack("bvo2", bvo2)
    pack("lnvg", g["ln_v_g"])
    pack("lnvb", g["ln_v_b"])
    pack("lnng", g["ln_n_g"])
    pack("lnnb", g["ln_n_b"])
    # LN beta folded into FFN biases: h = gelu(W1 @ (u + beta) + b1)
    # = gelu(W1 @ u + (b1 + W1 @ beta)); out = u + beta + d + b2
    pack("b2v", g["fv_b2"] + g["ln_v_b"])
    pack("b2n", g["fn_b2"] + g["ln_n_b"])
    pack("b1v", g["fv_b1"] + g["fv_w1"] @ g["ln_v_b"])
    pack("b1n", g["fn_b1"] + g["fn_w1"] @ g["ln_n_b"])

    common = {
        "wvo1": wvo1, "wvo2": wvo2,
        "w1v": np.ascontiguousarray(g["fv_w1"].T).astype(bf16),
        "w2v": np.ascontiguousarray(g["fv_w2"].T).astype(bf16),
        "w1n": np.ascontiguousarray(g["fn_w1"].T).astype(bf16),
        "w2n": np.ascontiguousarray(g["fn_w2"].T).astype(bf16),
        "vecs": vec_tab,
        "ones_d": np.ones((P, P), f32),
    }
    vTf = np.ascontiguousarray(g["verb_features"].T).astype(bf16)  # [E, 16384]
    nTf = np.ascontiguousarray(g["noun_features"].T).astype(bf16)
    in_maps = []
    for i in range(NCORES):
        cs = slice(i * B, (i + 1) * B)
        m = dict(common)
        m["vT"] = np.ascontiguousarray(vTf[:, cs])
        m["nT"] = np.ascontiguousarray(nTf[:, cs])
        in_maps.append(m)
    return in_maps


def kernel(**inputs):
    nc = _get_program()
    in_maps = _prepare_maps(inputs)
    res = run_bass_kernel_spmd(nc, in_maps, list(range(NCORES))).results
    verb = np.concatenate([res[i]["verb_out"] for i in range(NCORES)], axis=1)
    noun = np.concatenate([res[i]["noun_out"] for i in range(NCORES)], axis=1)
    return np.ascontiguousarray(verb.T), np.ascontiguousarray(noun.T)
